# revision 14
# baseline (speedup 1.0000x reference)
"""Multi-head attention (B=4, S=2048, D=1024, 16 heads x 64) on 8 trn2 cores.

Sharding: core c handles batch b = c//2 and head-group hg = c%2 (8 heads each,
i.e. columns hg*512:(hg+1)*512 of Wq/Wk/Wv and rows of Wo).  Each core returns
a partial output [S, D]; the host sums the two partials per batch and adds bo.

v7 (vs the v5 phase-serial baseline, HW-measured deltas):
  * single flat pool scope (no phase ExitStacks) + emission order
    [K-proj, Q-proj(0), V-proj, then per chunk: attention(c), Q-proj(c+1),
    out-proj(c)] so the Tile scheduler overlaps projection/out-projection PE
    work with the exp-bound attention windows.  PSUM: scores 2x[128,2,512]
    (4 banks) + zq 2x[128,512] (2) + shared proj/outproj pool (2) = 8.
  * exp split: every 4th score tile exps on DVE via the Schraudolph bf16-bit
    trick (tensor_scalar mult+add into int16 bitcast, ~1281ns/tile from PSUM);
    the rest take exact Exp on ScalarE (1147ns/tile).  Rel err 0.0139 vs the
    2e-2 gate (all-ScalarE: 0.0084; every-3rd: 0.0240 FAILS -- do not raise
    the DVE share).
  * PV lhsT padded 65 -> 128 columns (vb row stride 583, junk PSUM rows
    65:127 never read): NumWeights==128 enables FWL; HW microbench 262.9 ->
    227.5 ns/matmul, ~36us of PE across the 1024 PV matmuls.
  * fp8/DoubleRow was tried for the projections and REVERTED: e4m3 carries
    ~2.5% relative error per matmul output (error and signal both scale with
    sqrt(K)), which exp amplifies -> rel err 0.091.  Keep everything bf16.

Engine budget per core (HW-calibrated): PE ~305us busy (scores 67.5 row-pair
+ PV ~98 + projections 116.5), ScalarE ~220us exp, DVE ~190us
(exp share + evictions + normalize).  Measured wall ~415-445ns*1e3 with
~4% run-to-run machine drift.

build_nc(reps=N) wraps the body in a hardware For_i loop for the timing
harness; reps=1 emits the plain program used by kernel().
"""

import numpy as np
import ml_dtypes

import concourse.bass as bass
import concourse.tile as tile
from concourse import bacc, mybir
from concourse.bass_utils import run_bass_kernel_spmd

F32 = mybir.dt.float32
BF16 = mybir.dt.bfloat16
FP8 = mybir.dt.float8e4
I16 = mybir.dt.int16
ACT = mybir.ActivationFunctionType
ALU = mybir.AluOpType

D = 1024          # d_model
HH = 512          # heads-per-core * head_dim = 8 * 64
HD = 64           # head dim
NHL = 8           # heads per core
B, S_FULL = 4, 2048
N_CORES = 8

LOG2E = 1.4426950408889634
SCHRAUD_A = 0.125 * LOG2E * 128.0          # scores scale 1/8 folded in
SCHRAUD_B = 127.0 * 128.0 - 5.5            # C=5.5: max rel 3.3%, rms 2.1%
EXP_DVE_EVERY = 4    # every Nth score tile exps on DVE (Schraudolph)
WSCALE = 16.0        # host-side prescale of fp8 weights (dodges e4m3
                     # subnormals for ~N(0,1/32) entries); undone at eviction


def build_nc(S=S_FULL, reps=1):
    nc = bacc.Bacc("TRN2", target_bir_lowering=False, debug=False,
                   dynamic_dma_scratch_size=2048)

    xqT = nc.dram_tensor("xqT", [D, S], BF16, kind="ExternalInput").ap()
    xkT = nc.dram_tensor("xkT", [D, S], BF16, kind="ExternalInput").ap()
    xvT = nc.dram_tensor("xvT", [D, S], BF16, kind="ExternalInput").ap()
    wq = nc.dram_tensor("wq", [D, HH], BF16, kind="ExternalInput").ap()
    wk = nc.dram_tensor("wk", [D, HH], BF16, kind="ExternalInput").ap()
    wv = nc.dram_tensor("wv", [D, HH], BF16, kind="ExternalInput").ap()
    wo = nc.dram_tensor("wo", [HH, D], BF16, kind="ExternalInput").ap()
    bq = nc.dram_tensor("bq", [HH], F32, kind="ExternalInput").ap()
    bk = nc.dram_tensor("bk", [HH], F32, kind="ExternalInput").ap()
    bv = nc.dram_tensor("bv", [HH], F32, kind="ExternalInput").ap()
    out = nc.dram_tensor("out", [S, D], F32, kind="ExternalOutput").ap()

    NT = S // 512        # 512-token chunks
    NSK = S // 128       # 128-token key tiles
    NKT = D // 128       # 128-wide d_model tiles
    NKB = HH // 128      # 128-wide hidden tiles (head pairs)
    DD = HD + 1          # V head-group width (64 values + ones column)

    with tile.TileContext(nc) as tc:
        from contextlib import ExitStack

        rep_loop = tc.For_i(0, reps, 1) if reps > 1 else None
        if rep_loop is not None:
            rep_loop.__enter__()

        with ExitStack() as ctx:
            persist = ctx.enter_context(tc.tile_pool(name="persist", bufs=1))
            qt_sb = persist.tile([128, NKB, S], BF16, tag="qt")
            kt_sb = persist.tile([128, NKB, S], BF16, tag="kt")
            VBW = NHL * DD + 128 - DD   # pad: 128-wide FWL lhsT slices in-bounds
            vb_sb = persist.tile([128, NSK, VBW], BF16, tag="vb")
            zt_sb = persist.tile([128, NKB, S], BF16, tag="zt")
            wo_sb = persist.tile([128, NKB, D], BF16, tag="wo")
            wq_sb = persist.tile([128, NKT, HH], BF16, tag="wqs")
            wk_sb = persist.tile([128, NKT, HH], BF16, tag="wks")
            wv_sb = persist.tile([128, NKT, HH], BF16, tag="wvs")
            bq_sb = persist.tile([128, NKB], F32, tag="bq")
            bk_sb = persist.tile([128, NKB], F32, tag="bk")
            bvb_sb = persist.tile([128, HH], F32, tag="bvb")

            kxpool = ctx.enter_context(tc.tile_pool(name="kxpool", bufs=4))
            xpool = ctx.enter_context(tc.tile_pool(name="xpool", bufs=3))
            ptpool = ctx.enter_context(tc.tile_pool(name="ptpool", bufs=10))
            npool = ctx.enter_context(tc.tile_pool(name="npool", bufs=2))
            opool = ctx.enter_context(tc.tile_pool(name="opool", bufs=2))
            pp = ctx.enter_context(tc.tile_pool(name="pp", bufs=2, space="PSUM"))
            sppool = ctx.enter_context(tc.tile_pool(name="sp", bufs=2, space="PSUM"))
            zqpool = ctx.enter_context(tc.tile_pool(name="zq", bufs=1, space="PSUM"))

            nc.sync.dma_start(out=wk_sb,
                              in_=wk.rearrange("(kt p) n -> p kt n", p=128))
            nc.sync.dma_start(out=bq_sb, in_=bq.rearrange("(kb p) -> p kb", p=128))
            nc.sync.dma_start(out=bk_sb, in_=bk.rearrange("(kb p) -> p kb", p=128))
            bv_bcast_in = bass.AP(tensor=bv.tensor, offset=bv.offset,
                                  ap=[[0, 128], [1, HH]])
            nc.sync.dma_start(out=bvb_sb, in_=bv_bcast_in)
            nc.sync.dma_start(out=wq_sb,
                              in_=wq.rearrange("(kt p) n -> p kt n", p=128))
            nc.sync.dma_start(out=wv_sb,
                              in_=wv.rearrange("(kt p) n -> p kt n", p=128))
            nc.sync.dma_start(out=wo_sb, in_=wo.rearrange("(hb p) n -> p hb n", p=128))
            # ones columns of V~ (softmax denominator trick)
            ones_view = vb_sb[:, :, 0:NHL * DD].rearrange(
                "p s (h dd) -> p s h dd", dd=DD)[:, :, :, HD:HD + 1]
            nc.vector.memset(ones_view, 1.0)
            nc.vector.memset(vb_sb[:, :, NHL * DD:], 0.0)
            # preload the exp ACT table before the pipeline needs it
            warm = persist.tile([1, 1], BF16, tag="warm")
            nc.scalar.activation(warm, bq_sb[0:1, 0:1], ACT.Exp, scale=1.0)

            def load_x(xT, t, pool, nm, eng=None):
                xt = pool.tile([128, NKT, 512], BF16, tag="xt", name=nm)
                (eng or nc.sync).dma_start(
                    out=xt,
                    in_=xT.rearrange("(kt p) s -> p kt s", p=128)[:, :, t * 512:(t + 1) * 512])
                return xt

            def proj_group(w_sb, xt, dst, bias, t, kb):
                ps = pp.tile([128, 512], F32, tag="pp",
                             name=f"ps_{dst.name}_{t}_{kb}")
                for kt in range(NKT):
                    nc.tensor.matmul(
                        ps,
                        lhsT=w_sb[:, kt, kb * 128:(kb + 1) * 128],
                        rhs=xt[:, kt, :],
                        start=(kt == 0), stop=(kt == NKT - 1))
                nc.vector.tensor_scalar_add(
                    dst[:, kb, t * 512:(t + 1) * 512], ps,
                    bias[:, kb:kb + 1])

            def proj_qk(xT, w_sb, dst, bias, t):
                xt = load_x(xT, t, xpool, f"xt_{dst.name}_{t}")
                for kb in range(NKB):
                    proj_group(w_sb, xt, dst, bias, t, kb)

            def proj_v(t):
                xt = xpool.tile([128, NKT, 512], BF16, tag="xt",
                                name=f"xt_v_{t}")
                nc.sync.dma_start(
                    out=xt,
                    in_=xvT.rearrange("(kt p) s -> p kt s", p=128)[:, :, t * 512:(t + 1) * 512])
                for m in range(4):
                    ps = pp.tile([128, 512], F32, tag="pp", name=f"ps_v_{t}_{m}")
                    for kt in range(NKT):
                        nc.tensor.matmul(
                            ps,
                            lhsT=xt[:, kt, m * 128:(m + 1) * 128],
                            rhs=wv_sb[:, kt, :],
                            start=(kt == 0), stop=(kt == NKT - 1))
                    sk = t * 4 + m
                    vdst = vb_sb[:, sk, 0:NHL * DD].rearrange(
                        "p (h dd) -> p h dd", dd=DD)[:, :, 0:HD]
                    nc.vector.tensor_add(
                        vdst,
                        ps.rearrange("p (h d) -> p h d", d=HD),
                        bvb_sb.rearrange("p (h d) -> p h d", d=HD))

            exp_state = [0]

            def attn(c):
                BLK = 4
                NB = NSK // BLK
                for kb in range(NKB):
                    zps = [zqpool.tile([128, 512], F32, tag=f"z{d}",
                                       name=f"zps{d}_{kb}_{c}")
                           for d in range(2)]
                    pts = {}
                    for blk in range(NB + 1):
                        if blk < NB:
                            # scores burst: 64x128 row-pair mode
                            for sk in range(blk * BLK, (blk + 1) * BLK):
                                spt = sppool.tile([128, 2, 512], F32, tag="sp")
                                for d in range(2):
                                    nc.tensor.matmul(
                                        spt[:, d, :],
                                        lhsT=kt_sb[d * 64:(d + 1) * 64, kb,
                                                   sk * 128:(sk + 1) * 128],
                                        rhs=qt_sb[d * 64:(d + 1) * 64, kb,
                                                  c * 512:(c + 1) * 512],
                                        start=True, stop=True)
                                pt = ptpool.tile([128, 2, 512], BF16, tag="pt")
                                u = exp_state[0]
                                if EXP_DVE_EVERY and \
                                        u % EXP_DVE_EVERY == EXP_DVE_EVERY - 1:
                                    nc.vector.tensor_scalar(
                                        pt.bitcast(I16), spt,
                                        SCHRAUD_A, SCHRAUD_B,
                                        ALU.mult, ALU.add)
                                else:
                                    nc.scalar.activation(pt, spt, ACT.Exp,
                                                         scale=0.125)
                                exp_state[0] = u + 1
                                pts[sk] = pt
                        if blk >= 1:
                            # PV burst for previous block: 65-wide serial
                            for j in range((blk - 1) * BLK, blk * BLK):
                                pt = pts.pop(j)
                                for d in range(2):
                                    hh = 2 * kb + d
                                    nc.tensor.matmul(
                                        zps[d],
                                        lhsT=vb_sb[:, j, hh * DD:hh * DD + 128],
                                        rhs=pt[:, d, :],
                                        start=(j == 0), stop=(j == NSK - 1))

                    # normalize: z / denominator-row -> zt bf16
                    for d in range(2):
                        dcp = npool.tile([1, 512], F32, tag="dcp",
                                         name=f"dcp_{c}_{kb}_{d}")
                        nc.vector.tensor_copy(dcp, zps[d][HD:HD + 1, :])
                        rc = npool.tile([1, 512], F32, tag="rc",
                                        name=f"rc_{c}_{kb}_{d}")
                        nc.vector.reciprocal_approx_fast(rc, dcp)
                        bc = npool.tile([HD, 512], F32, tag="bc",
                                        name=f"bc_{c}_{kb}_{d}")
                        nc.gpsimd.partition_broadcast(bc, rc, channels=HD)
                        nc.vector.tensor_mul(
                            zt_sb[d * 64:(d + 1) * 64, kb, c * 512:(c + 1) * 512],
                            zps[d][0:HD, :], bc)

            def outproj(c):
                for t in range(4 * c, 4 * c + 4):
                    os_t = opool.tile([128, D], F32, tag="os", name=f"os_{t}")
                    for n in range(D // 512):
                        po = pp.tile([128, 512], F32, tag="pp",
                                     name=f"po_{t}_{n}")
                        for hb in range(NKB):
                            nc.tensor.matmul(
                                po,
                                lhsT=zt_sb[:, hb, t * 128:(t + 1) * 128],
                                rhs=wo_sb[:, hb, n * 512:(n + 1) * 512],
                                start=(hb == 0), stop=(hb == NKB - 1))
                        nc.vector.tensor_copy(os_t[:, n * 512:(n + 1) * 512], po)
                    nc.sync.dma_start(out=out[t * 128:(t + 1) * 128, :], in_=os_t)

            # ---------------- emission (= scheduler priority) ----------------
            kxts = [load_x(xkT, t, kxpool, f"xt_k_{t}") for t in range(NT)]
            xq0 = load_x(xqT, 0, xpool, "xt_q_0")
            for kb in range(NKB):
                for t in range(NT):
                    proj_group(wk_sb, kxts[t], kt_sb, bk_sb, t, kb)
                proj_group(wq_sb, xq0, qt_sb, bq_sb, 0, kb)
            for t in range(NT):
                proj_v(t)
            for c in range(NT):
                attn(c)
                if c + 1 < NT:
                    proj_qk(xqT, wq_sb, qt_sb, bq_sb, c + 1)
                outproj(c)

        if rep_loop is not None:
            rep_loop.__exit__(None, None, None)

    nc.compile()
    return nc


_NC_CACHE = {}


def _get_nc(S=S_FULL, reps=1):
    key = (S, reps)
    if key not in _NC_CACHE:
        _NC_CACHE[key] = build_nc(S, reps=reps)
    return _NC_CACHE[key]


def make_in_maps(query, key, value, Wq, bq, Wk, bk, Wv, bv, Wo, bo):
    """Shard full inputs into 8 per-core input dicts (bf16 operands)."""
    bf = lambda a: np.ascontiguousarray(np.asarray(a, dtype=np.float32)).astype(ml_dtypes.bfloat16)
    f8 = lambda a: np.ascontiguousarray(np.asarray(a, dtype=np.float32)).astype(ml_dtypes.float8_e4m3)
    f32 = lambda a: np.ascontiguousarray(np.asarray(a, dtype=np.float32))
    in_maps = []
    for core in range(N_CORES):
        b, hg = core // 2, core % 2
        sl = slice(hg * HH, (hg + 1) * HH)
        in_maps.append({
            "xqT": bf(np.asarray(query)[b].T),
            "xkT": bf(np.asarray(key)[b].T),
            "xvT": bf(np.asarray(value)[b].T),
            "wq": bf(np.asarray(Wq)[:, sl]),
            "wk": bf(np.asarray(Wk)[:, sl]),
            "wv": bf(np.asarray(Wv)[:, sl]),
            "wo": bf(np.asarray(Wo)[sl, :]),
            "bq": f32(np.asarray(bq)[sl]),
            "bk": f32(np.asarray(bk)[sl]),
            "bv": f32(np.asarray(bv)[sl]),
        })
    return in_maps


def kernel(query, key, value, Wq, bq, Wk, bk, Wv, bv, Wo, bo, **run_kwargs):
    nc = _get_nc(S_FULL)
    in_maps = make_in_maps(query, key, value, Wq, bq, Wk, bk, Wv, bv, Wo, bo)
    res = run_bass_kernel_spmd(nc, in_maps, core_ids=list(range(N_CORES)),
                               **run_kwargs)
    bo_np = np.asarray(bo, dtype=np.float32)
    outs = [np.asarray(r["out"], dtype=np.float32) for r in res.results]
    full = np.stack([outs[2 * b] + outs[2 * b + 1] + bo_np for b in range(B)])
    return full.astype(np.float32)
